# revision 10
# baseline (speedup 1.0000x reference)
"""PEER / product-key MoE routing kernel for Trainium2 (8 NeuronCores).

Data-parallel over tokens: each core takes 256 of the 2048 tokens plus a
full replica of the (bf16-packed) expert tables in DRAM. Routing runs in
fp32 (expert selection matches the fp32 reference exactly); only the
expert tables and the token activations used in the expert inner products
are bf16.

The kernel is organized as a per-head software pipeline so the expert-row
gather stream — the critical resource, bounded by the Pool engine's
~1.5us-per-128-row indirect-DMA cadence — starts as soon as head 0 of the
routing is resolved (~10us in) instead of after all routing (~75us):

  per head m (8 heads):
    PE:   q projection for head m (4 fp32 matmuls into PSUM),
          s1/s2 score matmuls per token-block (fp32, the two key halves
          live on partition ranges 0-63 / 64-127)
    DVE:  exact two-stage top-8 (max8/max_index), winner sub-key resolve
          via an is_equal one-hot reduction, per-head softmax
    Pool: 16 indirect-DMA gathers (8 slots x 2 token blocks), each
          fetching 128 expert rows of 2KB (w_down|w_up packed)
    DVE:  fused inner product per slot: scalar_tensor_tensor with
          accum_out (one pass over [128,512] bf16 -> fp32 accumulator)
    ACT:  relu, exp, and diag(va) builds via activation-with-scale
    PE:   PSUM-accumulated diag(va) @ w_up_row combine matmuls

HW-measured: ~237us (baseline was ~282us). The remaining time is the
Pool-engine indirect-DMA cadence: 128 gathers x ~1.5us (994ns SWDGE ucode
launch + ~0.34ns/descriptor + ~310ns issue gap), which bounds the stream
at ~190us regardless of row size (1KB rows gather at the same cadence).
dma_gather (InstDMAGatherAnt) batches descriptors at full DMA bandwidth
(~390GB/s measured) and its int16-index limit is solvable with a two-pass
lo/hi gather (hi indices = id XOR 0x8000; negative positions are skipped
without corrupting), but the two passes WAW-serialize per page tile and
need cross-call pipelining to win; see kernel_dg.py for the working-but-
serialized attempt (469us, correct).
"""

import numpy as np

import concourse.bass as bass
import concourse.mybir as mybir
from concourse import bacc
from concourse.bass import IndirectOffsetOnAxis
from concourse.tile import TileContext
from concourse.bass_utils import run_bass_kernel_spmd

N_CORES = 8
N_HEADS = 8
D_KEYS = 128
HALF = 64
N_KEYS = 256
TOP_K = 8
D = 512
B = 2048           # total tokens
BC = B // N_CORES  # tokens per core (256)
TB = BC // 128     # token blocks per core (2)
F32 = mybir.dt.float32
U16 = mybir.dt.uint16
I32 = mybir.dt.int32
BF16 = mybir.dt.bfloat16
X = mybir.AxisListType.X
OP = mybir.AluOpType
AF = mybir.ActivationFunctionType


def build_nc():
    nc = bacc.Bacc("TRN2", target_bir_lowering=False)

    xt_d = nc.dram_tensor("xt", [128, 4, BC], F32, kind="ExternalInput")
    xbf_d = nc.dram_tensor("xbf", [128, TB, D], BF16, kind="ExternalInput")
    wqh0_d = nc.dram_tensor("wqh0", [128, 4, 128], F32, kind="ExternalInput")
    wqr_d = nc.dram_tensor("wqr", [128, 4, 896], F32, kind="ExternalInput")
    bqp_d = nc.dram_tensor("bqp", [D_KEYS, N_HEADS], F32, kind="ExternalInput")
    kp1_d = nc.dram_tensor("kp1", [HALF, N_HEADS, N_KEYS], F32, kind="ExternalInput")
    kp2_d = nc.dram_tensor("kp2", [HALF, N_HEADS, N_KEYS], F32, kind="ExternalInput")
    wb_d = nc.dram_tensor("wb", [N_KEYS * N_KEYS, 2 * D], BF16,
                          kind="ExternalInput")
    id01_d = nc.dram_tensor("id01", [128, 128], BF16, kind="ExternalInput")
    iota8_d = nc.dram_tensor("iota8", [128, 8], U16, kind="ExternalInput")
    out_d = nc.dram_tensor("out", [BC, D], F32, kind="ExternalOutput")

    with TileContext(nc) as tc:
        with (
            tc.tile_pool(name="const", bufs=1) as cpool,
            tc.tile_pool(name="qh", bufs=3) as qhpool,
            tc.tile_pool(name="psq", bufs=2, space="PSUM") as psq,
            tc.tile_pool(name="pss", bufs=2, space="PSUM") as pss,
            tc.tile_pool(name="sc", bufs=4) as scpool,
            tc.tile_pool(name="tk", bufs=6) as tkpool,
            tc.tile_pool(name="st2", bufs=1) as st2,
            tc.tile_pool(name="wbp", bufs=44) as wbp,
            tc.tile_pool(name="scr", bufs=6) as scrp,
            tc.tile_pool(name="dgp", bufs=6) as dgp,
            tc.tile_pool(name="pacc", bufs=2, space="PSUM") as paccp,
            tc.tile_pool(name="accp", bufs=2) as accp,
        ):
            # ---- constant loads: one DMA per tensor (host pre-packed to
            # partition-major layouts), critical tensors first ----
            wq_all = cpool.tile([128, 4, N_HEADS * D_KEYS], F32, tag="wq")
            nc.sync.dma_start(out=wq_all[:, :, 0:128], in_=wqh0_d[:, :, :])
            xt_all = cpool.tile([128, 4, BC], F32, tag="xt")
            nc.scalar.dma_start(out=xt_all[:], in_=xt_d[:, :, :])
            # kp1 on partitions 0-63, kp2 on partitions 64-127 (so the score
            # matmuls' lhsT/rhs base partitions match)
            kp_sb = cpool.tile([128, N_HEADS, N_KEYS], F32, tag="kp")
            nc.sync.dma_start(out=kp_sb[0:HALF, :, :], in_=kp1_d[:, :, :])
            nc.scalar.dma_start(out=kp_sb[HALF:128, :, :], in_=kp2_d[:, :, :])
            bqp_sb = cpool.tile([D_KEYS, N_HEADS], F32, tag="bqp")
            nc.sync.dma_start(out=bqp_sb[:], in_=bqp_d[:, :])
            iota8 = cpool.tile([128, 8], U16, tag="iota8")
            nc.sync.dma_start(out=iota8[:], in_=iota8_d[:, :])
            xbf_all = cpool.tile([128, TB, D], BF16, tag="xbf")
            nc.scalar.dma_start(out=xbf_all[:], in_=xbf_d[:, :, :])
            id01_sb = cpool.tile([128, 128], BF16, tag="id01")
            nc.scalar.dma_start(out=id01_sb[:], in_=id01_d[:, :])
            nc.sync.dma_start(out=wq_all[:, :, 128:N_HEADS * D_KEYS],
                              in_=wqr_d[:, :, :])
            wq_sb = [wq_all[:, k, :] for k in range(4)]
            xt_sb = [xt_all[:, k, :] for k in range(4)]
            xbf_sb = [xbf_all[:, tb, :] for tb in range(TB)]

            inner = {}
            pacc = {}
            for tb in range(TB):
                inner[tb] = st2.tile([128, 64], F32, tag=f"inner{tb}",
                                     name=f"inner{tb}")
                pacc[tb] = paccp.tile([128, D], F32, tag=f"pacc{tb}",
                                      name=f"pacc{tb}")

            # ---- per-head software pipeline ----
            # route(m) resolves head m's expert ids + softmax weights;
            # gathers(m) issues the 16 indirect DMAs; compute(m) does the
            # inner products and combine. Emission order interleaves
            # route(m+1) between gathers(m) and compute(m) so the DVE
            # queue resolves the next head's indices before it sinks into
            # head m's inner products — keeping the Pool gather stream fed.
            def route(m):
                # q projection for head m: qh[128 features, 256 tokens].
                # Feature rows 0-63 are half-1 (c), 64-127 are half-2.
                ps = psq.tile([128, BC], F32, tag="psq")
                for k in range(4):
                    nc.tensor.matmul(
                        out=ps[:],
                        lhsT=wq_sb[k][:, m * 128:(m + 1) * 128],
                        rhs=xt_sb[k],
                        start=(k == 0),
                        stop=(k == 3),
                    )
                qh = qhpool.tile([128, BC], F32, tag="qh", name=f"qh{m}")
                nc.vector.tensor_scalar(
                    out=qh[:], in0=ps[:],
                    scalar1=bqp_sb[:, m:m + 1], scalar2=None, op0=OP.add,
                )

                routed = []
                for tb in range(TB):
                    tsl = slice(tb * 128, (tb + 1) * 128)
                    s1t = tkpool.tile([128, 8], F32, tag="s1t")
                    s2t = tkpool.tile([128, 8], F32, tag="s2t")
                    i1 = tkpool.tile([128, 8], U16, tag="i1")
                    i2 = tkpool.tile([128, 8], U16, tag="i2")
                    for half, (st_, ix) in enumerate(((s1t, i1), (s2t, i2))):
                        hsl = slice(half * HALF, (half + 1) * HALF)
                        pscore = pss.tile([128, N_KEYS], F32, tag="pss")
                        nc.tensor.matmul(
                            out=pscore[:],
                            lhsT=qh[hsl, tsl],
                            rhs=kp_sb[hsl, m, :],
                            start=True, stop=True,
                        )
                        s_sb = scpool.tile([128, N_KEYS], F32, tag="s_sb")
                        nc.scalar.copy(out=s_sb[:], in_=pscore[:])
                        nc.vector.max(out=st_[:], in_=s_sb[:])
                        nc.vector.max_index(
                            out=ix[:], in_max=st_[:], in_values=s_sb[:])

                    # stage-2: 8x8 combo scores, top-8, winner resolve
                    cs = tkpool.tile([128, 64], F32, tag="cs")
                    nc.vector.tensor_tensor(
                        out=cs[:, :].rearrange("p (a b) -> p a b", a=8),
                        in0=s1t[:, :].unsqueeze(2).to_broadcast([128, 8, 8]),
                        in1=s2t[:, :].unsqueeze(1).to_broadcast([128, 8, 8]),
                        op=OP.add,
                    )
                    v8 = tkpool.tile([128, 8], F32, tag="v8")
                    n8 = tkpool.tile([128, 8], U16, tag="n8")
                    nc.vector.max(out=v8[:], in_=cs[:])
                    nc.vector.max_index(out=n8[:], in_max=v8[:], in_values=cs[:])
                    k1 = tkpool.tile([128, 8], U16, tag="k1")
                    nc.vector.tensor_scalar(
                        out=k1[:], in0=n8[:], scalar1=3, scalar2=None,
                        op0=OP.logical_shift_right)
                    k2 = tkpool.tile([128, 8], U16, tag="k2")
                    nc.vector.tensor_scalar(
                        out=k2[:], in0=n8[:], scalar1=7, scalar2=None,
                        op0=OP.bitwise_and)
                    sels = []
                    for sidx, (kk, ix) in enumerate(((k1, i1), (k2, i2))):
                        eq = tkpool.tile([128, 64], U16, tag="eq")
                        nc.vector.tensor_tensor(
                            out=eq[:, :].rearrange("p (j k) -> p j k", j=8),
                            in0=kk[:, :].unsqueeze(2).to_broadcast([128, 8, 8]),
                            in1=iota8[:, :].unsqueeze(1).to_broadcast([128, 8, 8]),
                            op=OP.is_equal)
                        prod = tkpool.tile([128, 64], U16, tag="prod")
                        nc.vector.tensor_tensor(
                            out=prod[:, :].rearrange("p (j k) -> p j k", j=8),
                            in0=eq[:, :].rearrange("p (j k) -> p j k", j=8),
                            in1=ix[:, :].unsqueeze(1).to_broadcast([128, 8, 8]),
                            op=OP.mult)
                        sel = tkpool.tile([128, 8], U16, tag=f"sel{sidx}",
                                          name=f"sel{sidx}_{m}_{tb}")
                        with nc.allow_low_precision(
                                reason="one-hot uint16 sum, values <= 255"):
                            nc.vector.reduce_sum(
                                out=sel[:],
                                in_=prod[:, :].rearrange("p (j k) -> p j k", j=8),
                                axis=X)
                        sels.append(sel)
                    idx16 = tkpool.tile([128, 8], U16, tag="idx16")
                    nc.vector.tensor_scalar(
                        out=idx16[:], in0=sels[0][:], scalar1=256, scalar2=None,
                        op0=OP.mult)
                    nc.vector.tensor_tensor(
                        out=idx16[:], in0=idx16[:], in1=sels[1][:], op=OP.add)
                    idx32 = tkpool.tile([128, 8], I32, tag="idx32",
                                        name=f"idx32_{m}_{tb}")
                    nc.vector.tensor_copy(out=idx32[:], in_=idx16[:])

                    # per-head softmax over the encoded top-8 scores
                    rmax = tkpool.tile([128, 1], F32, tag="rmax")
                    nc.vector.reduce_max(out=rmax[:], in_=v8[:], axis=X)
                    ex = tkpool.tile([128, 8], F32, tag="ex")
                    nc.vector.tensor_scalar(
                        out=ex[:], in0=v8[:], scalar1=rmax[:, 0:1], scalar2=None,
                        op0=OP.subtract)
                    nc.scalar.activation(out=ex[:], in_=ex[:], func=AF.Exp)
                    rsum = tkpool.tile([128, 1], F32, tag="rsum")
                    nc.vector.reduce_sum(out=rsum[:], in_=ex[:], axis=X)
                    rinv = tkpool.tile([128, 1], F32, tag="rinv")
                    nc.vector.reciprocal(out=rinv[:], in_=rsum[:])
                    w8 = tkpool.tile([128, 8], F32, tag="w8",
                                     name=f"w8_{m}_{tb}")
                    nc.vector.tensor_scalar(
                        out=w8[:], in0=ex[:], scalar1=rinv[:, 0:1], scalar2=None,
                        op0=OP.mult)
                    routed.append((idx32, w8))
                return routed

            def gathers(m, routed):
                pages = {}
                order = [(j, tb) for j in range(8) for tb in range(TB)]
                if m == N_HEADS - 1:
                    order = [(j, tb) for tb in range(TB) for j in range(8)]
                for j, tb in order:
                        idx32, _ = routed[tb]
                        col = m * 8 + j
                        page = wbp.tile([128, 2 * D], BF16, tag="wbpage",
                                        name=f"pg{tb}_{col}")
                        nc.gpsimd.indirect_dma_start(
                            out=page[:], out_offset=None,
                            in_=wb_d[:, :],
                            in_offset=IndirectOffsetOnAxis(
                                ap=idx32[:, j:j + 1], axis=0),
                        )
                        pages[(tb, j)] = page
                return pages

            def compute(m, routed, pages):
                gs = slice(m * 8, (m + 1) * 8)
                order = [(j, tb) for j in range(8) for tb in range(TB)]
                if m == N_HEADS - 1:
                    order = [(j, tb) for tb in range(TB) for j in range(8)]
                for j, tb in order:
                        col = m * 8 + j
                        dummy = scrp.tile([128, 1], BF16, tag="scr")
                        nc.vector.scalar_tensor_tensor(
                            out=dummy.broadcast_to([128, D]),
                            in0=pages[(tb, j)][:, 0:D], scalar=1.0,
                            in1=xbf_sb[tb], op0=OP.mult, op1=OP.mult,
                            accum_out=inner[tb][:, col:col + 1],
                        )
                for tb in range(TB):
                    _, w8 = routed[tb]
                    rl8 = tkpool.tile([128, 8], F32, tag="rl8")
                    nc.scalar.activation(
                        out=rl8[:], in_=inner[tb][:, gs], func=AF.Relu)
                    va8 = tkpool.tile([128, 8], F32, tag="va8",
                                      name=f"va8_{m}_{tb}")
                    nc.vector.tensor_tensor(
                        out=va8[:], in0=rl8[:], in1=w8[:], op=OP.mult)
                    for j in range(8):
                        col = m * 8 + j
                        diag = dgp.tile([128, 128], BF16, tag="diag")
                        nc.scalar.activation(
                            out=diag[:], in_=id01_sb[:], func=AF.Copy,
                            scale=va8[:, j:j + 1])
                        nc.tensor.matmul(
                            out=pacc[tb][:], lhsT=diag[:],
                            rhs=pages[(tb, j)][:, D:2 * D],
                            start=(col == 0), stop=(col == 63))

            routed_m = route(0)
            for m in range(N_HEADS):
                pages_m = gathers(m, routed_m)
                routed_next = route(m + 1) if m + 1 < N_HEADS else None
                compute(m, routed_m, pages_m)
                routed_m = routed_next

            for tb in range(TB):
                acc_sb = accp.tile([128, D], F32, tag=f"acc{tb}",
                                   name=f"acc{tb}")
                nc.vector.tensor_copy(out=acc_sb[:], in_=pacc[tb][:])
                nc.sync.dma_start(
                    out=out_d[tb * 128:(tb + 1) * 128, :], in_=acc_sb[:])

    nc.compile()
    return nc


_NC_CACHE = None


def _get_nc():
    global _NC_CACHE
    if _NC_CACHE is None:
        _NC_CACHE = build_nc()
    return _NC_CACHE


def _prep_in_maps(inputs):
    import ml_dtypes
    q = np.ascontiguousarray(np.asarray(inputs["queries"], dtype=np.float32))
    Wq = np.ascontiguousarray(np.asarray(inputs["Wq"], dtype=np.float32))
    bq = np.asarray(inputs["bq"], dtype=np.float32)
    keys = np.asarray(inputs["keys"], dtype=np.float32)
    wd = np.asarray(inputs["w_down"], dtype=np.float32)
    wu = np.asarray(inputs["w_up"], dtype=np.float32)
    wb = np.ascontiguousarray(
        np.concatenate([wd, wu], axis=1).astype(ml_dtypes.bfloat16))
    id01 = np.eye(128, dtype=np.float32).astype(ml_dtypes.bfloat16)
    iota8 = np.tile(np.arange(8, dtype=np.uint16), (128, 1))

    x = q.reshape(B, D)
    # bqp[p, m] = bq[m*128 + p]
    bqp = np.ascontiguousarray(bq.reshape(N_HEADS, D_KEYS).T)
    # kp{1,2}[c, m, n] = keys[m, half, n, c]
    kp1 = np.ascontiguousarray(keys[:, 0].transpose(2, 0, 1))
    kp2 = np.ascontiguousarray(keys[:, 1].transpose(2, 0, 1))
    # wq repacked [p, k, f] = Wq[k*128+p, f]; head-0 columns split out
    wq_r = Wq.reshape(4, 128, N_HEADS * D_KEYS).transpose(1, 0, 2)
    wqh0 = np.ascontiguousarray(wq_r[:, :, 0:128])
    wqr = np.ascontiguousarray(wq_r[:, :, 128:])

    in_maps = []
    for c in range(N_CORES):
        xc = x[c * BC:(c + 1) * BC]
        # xt [p, k, t] = xc.T[k*128+p, t]
        xt_r = np.ascontiguousarray(
            xc.T.reshape(4, 128, BC).transpose(1, 0, 2))
        xbf_r = np.ascontiguousarray(
            xc.reshape(TB, 128, D).transpose(1, 0, 2)
            .astype(ml_dtypes.bfloat16))
        in_maps.append({
            "xt": xt_r,
            "xbf": xbf_r,
            "wqh0": wqh0,
            "wqr": wqr,
            "bqp": bqp,
            "kp1": kp1,
            "kp2": kp2,
            "wb": wb,
            "id01": id01,
            "iota8": iota8,
        })
    return in_maps


def run(inputs, trace=False):
    """Run on 8 NeuronCores; returns (out [2,1024,512], BassKernelResults)."""
    nc = _get_nc()
    in_maps = _prep_in_maps(inputs)
    res = run_bass_kernel_spmd(
        nc, in_maps, core_ids=list(range(N_CORES)), trace=trace)
    out = np.concatenate(
        [res.results[c]["out"] for c in range(N_CORES)], axis=0)
    return out.reshape(2, 1024, D), res


def kernel(**inputs) -> np.ndarray:
    out, _ = run(inputs, trace=False)
    return out


# revision 11
# speedup vs baseline: 1.0183x; 1.0183x over previous
"""PEER / product-key MoE routing kernel for Trainium2 (8 NeuronCores).

Data-parallel over tokens: each core takes 256 of the 2048 tokens plus a
full replica of the (bf16-packed) expert tables in DRAM. Routing runs in
fp32 (expert selection matches the fp32 reference exactly); only the
expert tables and the token activations used in the expert inner products
are bf16.

The kernel is organized as a per-head software pipeline so the expert-row
gather stream — the critical resource, bounded by the Pool engine's
~1.5us-per-128-row indirect-DMA cadence — starts as soon as head 0 of the
routing is resolved (~10us in) instead of after all routing (~75us):

  per head m (8 heads):
    PE:   q projection for head m (4 fp32 matmuls into PSUM),
          s1/s2 score matmuls per token-block (fp32, the two key halves
          live on partition ranges 0-63 / 64-127)
    DVE:  exact two-stage top-8 (max8/max_index), winner sub-key resolve
          via an is_equal one-hot reduction, per-head softmax
    Pool: 16 indirect-DMA gathers (8 slots x 2 token blocks), each
          fetching 128 expert rows of 2KB (w_down|w_up packed)
    DVE:  fused inner product per slot: scalar_tensor_tensor with
          accum_out (one pass over [128,512] bf16 -> fp32 accumulator)
    ACT:  relu, exp, and diag(va) builds via activation-with-scale
    PE:   PSUM-accumulated diag(va) @ w_up_row combine matmuls

HW-measured: ~237us (baseline was ~282us). The remaining time is the
Pool-engine indirect-DMA cadence: 128 gathers x ~1.5us (994ns SWDGE ucode
launch + ~0.34ns/descriptor + ~310ns issue gap), which bounds the stream
at ~190us regardless of row size (1KB rows gather at the same cadence).
dma_gather (InstDMAGatherAnt) batches descriptors at full DMA bandwidth
(~390GB/s measured) and its int16-index limit is solvable with a two-pass
lo/hi gather (hi indices = id XOR 0x8000; negative positions are skipped
without corrupting), but the two passes WAW-serialize per page tile and
need cross-call pipelining to win; see kernel_dg.py for the working-but-
serialized attempt (469us, correct).
"""

import numpy as np

import concourse.bass as bass
import concourse.mybir as mybir
from concourse import bacc
from concourse.bass import IndirectOffsetOnAxis
from concourse.tile import TileContext
from concourse.bass_utils import run_bass_kernel_spmd

N_CORES = 8
N_HEADS = 8
D_KEYS = 128
HALF = 64
N_KEYS = 256
TOP_K = 8
D = 512
B = 2048           # total tokens
BC = B // N_CORES  # tokens per core (256)
TB = BC // 128     # token blocks per core (2)
F32 = mybir.dt.float32
U16 = mybir.dt.uint16
I32 = mybir.dt.int32
BF16 = mybir.dt.bfloat16
X = mybir.AxisListType.X
OP = mybir.AluOpType
AF = mybir.ActivationFunctionType


def build_nc():
    nc = bacc.Bacc("TRN2", target_bir_lowering=False)

    xt_d = nc.dram_tensor("xt", [128, 4, BC], F32, kind="ExternalInput")
    xbf_d = nc.dram_tensor("xbf", [128, TB, D], BF16, kind="ExternalInput")
    wqh0_d = nc.dram_tensor("wqh0", [128, 4, 128], F32, kind="ExternalInput")
    wqr_d = nc.dram_tensor("wqr", [128, 4, 896], F32, kind="ExternalInput")
    bqp_d = nc.dram_tensor("bqp", [D_KEYS, N_HEADS], F32, kind="ExternalInput")
    kp1_d = nc.dram_tensor("kp1", [HALF, N_HEADS, N_KEYS], F32, kind="ExternalInput")
    kp2_d = nc.dram_tensor("kp2", [HALF, N_HEADS, N_KEYS], F32, kind="ExternalInput")
    wb_d = nc.dram_tensor("wb", [N_KEYS * N_KEYS, 2 * D], BF16,
                          kind="ExternalInput")
    id01_d = nc.dram_tensor("id01", [128, 128], BF16, kind="ExternalInput")
    iota8_d = nc.dram_tensor("iota8", [128, 8], U16, kind="ExternalInput")
    out_d = nc.dram_tensor("out", [BC, D], F32, kind="ExternalOutput")

    with TileContext(nc) as tc:
        with (
            tc.tile_pool(name="const", bufs=1) as cpool,
            tc.tile_pool(name="qh", bufs=3) as qhpool,
            tc.tile_pool(name="psq", bufs=2, space="PSUM") as psq,
            tc.tile_pool(name="pss", bufs=2, space="PSUM") as pss,
            tc.tile_pool(name="sc", bufs=4) as scpool,
            tc.tile_pool(name="tk", bufs=6) as tkpool,
            tc.tile_pool(name="st2", bufs=1) as st2,
            tc.tile_pool(name="wbp", bufs=44) as wbp,
            tc.tile_pool(name="scr", bufs=6) as scrp,
            tc.tile_pool(name="dgp", bufs=6) as dgp,
            tc.tile_pool(name="pacc", bufs=2, space="PSUM") as paccp,
            tc.tile_pool(name="accp", bufs=2) as accp,
        ):
            # ---- constant loads: one DMA per tensor (host pre-packed to
            # partition-major layouts), critical tensors first ----
            wq_all = cpool.tile([128, 4, N_HEADS * D_KEYS], F32, tag="wq")
            nc.sync.dma_start(out=wq_all[:, :, 0:128], in_=wqh0_d[:, :, :])
            xt_all = cpool.tile([128, 4, BC], F32, tag="xt")
            nc.scalar.dma_start(out=xt_all[:], in_=xt_d[:, :, :])
            # kp1 on partitions 0-63, kp2 on partitions 64-127 (so the score
            # matmuls' lhsT/rhs base partitions match)
            kp_sb = cpool.tile([128, N_HEADS, N_KEYS], F32, tag="kp")
            nc.sync.dma_start(out=kp_sb[0:HALF, :, :], in_=kp1_d[:, :, :])
            nc.scalar.dma_start(out=kp_sb[HALF:128, :, :], in_=kp2_d[:, :, :])
            bqp_sb = cpool.tile([D_KEYS, N_HEADS], F32, tag="bqp")
            nc.sync.dma_start(out=bqp_sb[:], in_=bqp_d[:, :])
            iota8 = cpool.tile([128, 8], U16, tag="iota8")
            nc.sync.dma_start(out=iota8[:], in_=iota8_d[:, :])
            xbf_all = cpool.tile([128, TB, D], BF16, tag="xbf")
            nc.scalar.dma_start(out=xbf_all[:], in_=xbf_d[:, :, :])
            id01_sb = cpool.tile([128, 128], BF16, tag="id01")
            nc.scalar.dma_start(out=id01_sb[:], in_=id01_d[:, :])
            nc.sync.dma_start(out=wq_all[:, :, 128:N_HEADS * D_KEYS],
                              in_=wqr_d[:, :, :])
            wq_sb = [wq_all[:, k, :] for k in range(4)]
            xt_sb = [xt_all[:, k, :] for k in range(4)]
            xbf_sb = [xbf_all[:, tb, :] for tb in range(TB)]

            inner = {}
            pacc = {}
            for tb in range(TB):
                inner[tb] = st2.tile([128, 64], F32, tag=f"inner{tb}",
                                     name=f"inner{tb}")
                pacc[tb] = paccp.tile([128, D], F32, tag=f"pacc{tb}",
                                      name=f"pacc{tb}")

            # ---- per-head software pipeline ----
            # route(m) resolves head m's expert ids + softmax weights;
            # gathers(m) issues the 16 indirect DMAs; compute(m) does the
            # inner products and combine. Emission order interleaves
            # route(m+1) between gathers(m) and compute(m) so the DVE
            # queue resolves the next head's indices before it sinks into
            # head m's inner products — keeping the Pool gather stream fed.
            def route(m):
                # q projection for head m: qh[128 features, 256 tokens].
                # Feature rows 0-63 are half-1 (c), 64-127 are half-2.
                ps = psq.tile([128, BC], F32, tag="psq")
                for k in range(4):
                    nc.tensor.matmul(
                        out=ps[:],
                        lhsT=wq_sb[k][:, m * 128:(m + 1) * 128],
                        rhs=xt_sb[k],
                        start=(k == 0),
                        stop=(k == 3),
                    )
                qh = qhpool.tile([128, BC], F32, tag="qh", name=f"qh{m}")
                nc.vector.tensor_scalar(
                    out=qh[:], in0=ps[:],
                    scalar1=bqp_sb[:, m:m + 1], scalar2=None, op0=OP.add,
                )

                routed = []
                for tb in range(TB):
                    tsl = slice(tb * 128, (tb + 1) * 128)
                    s1t = tkpool.tile([128, 8], F32, tag="s1t")
                    s2t = tkpool.tile([128, 8], F32, tag="s2t")
                    i1 = tkpool.tile([128, 8], U16, tag="i1")
                    i2 = tkpool.tile([128, 8], U16, tag="i2")
                    for half, (st_, ix) in enumerate(((s1t, i1), (s2t, i2))):
                        hsl = slice(half * HALF, (half + 1) * HALF)
                        pscore = pss.tile([128, N_KEYS], F32, tag="pss")
                        nc.tensor.matmul(
                            out=pscore[:],
                            lhsT=qh[hsl, tsl],
                            rhs=kp_sb[hsl, m, :],
                            start=True, stop=True,
                        )
                        s_sb = scpool.tile([128, N_KEYS], F32, tag="s_sb")
                        nc.scalar.copy(out=s_sb[:], in_=pscore[:])
                        nc.vector.max(out=st_[:], in_=s_sb[:])
                        nc.vector.max_index(
                            out=ix[:], in_max=st_[:], in_values=s_sb[:])

                    # stage-2: 8x8 combo scores, top-8, winner resolve
                    cs = tkpool.tile([128, 64], F32, tag="cs")
                    nc.vector.tensor_tensor(
                        out=cs[:, :].rearrange("p (a b) -> p a b", a=8),
                        in0=s1t[:, :].unsqueeze(2).to_broadcast([128, 8, 8]),
                        in1=s2t[:, :].unsqueeze(1).to_broadcast([128, 8, 8]),
                        op=OP.add,
                    )
                    v8 = tkpool.tile([128, 8], F32, tag="v8")
                    n8 = tkpool.tile([128, 8], U16, tag="n8")
                    nc.vector.max(out=v8[:], in_=cs[:])
                    nc.vector.max_index(out=n8[:], in_max=v8[:], in_values=cs[:])
                    k1 = tkpool.tile([128, 8], U16, tag="k1")
                    nc.vector.tensor_scalar(
                        out=k1[:], in0=n8[:], scalar1=3, scalar2=None,
                        op0=OP.logical_shift_right)
                    k2 = tkpool.tile([128, 8], U16, tag="k2")
                    nc.vector.tensor_scalar(
                        out=k2[:], in0=n8[:], scalar1=7, scalar2=None,
                        op0=OP.bitwise_and)
                    sels = []
                    for sidx, (kk, ix) in enumerate(((k1, i1), (k2, i2))):
                        eq = tkpool.tile([128, 64], U16, tag="eq")
                        nc.vector.tensor_tensor(
                            out=eq[:, :].rearrange("p (j k) -> p j k", j=8),
                            in0=kk[:, :].unsqueeze(2).to_broadcast([128, 8, 8]),
                            in1=iota8[:, :].unsqueeze(1).to_broadcast([128, 8, 8]),
                            op=OP.is_equal)
                        prod = tkpool.tile([128, 64], U16, tag="prod")
                        nc.vector.tensor_tensor(
                            out=prod[:, :].rearrange("p (j k) -> p j k", j=8),
                            in0=eq[:, :].rearrange("p (j k) -> p j k", j=8),
                            in1=ix[:, :].unsqueeze(1).to_broadcast([128, 8, 8]),
                            op=OP.mult)
                        sel = tkpool.tile([128, 8], U16, tag=f"sel{sidx}",
                                          name=f"sel{sidx}_{m}_{tb}")
                        with nc.allow_low_precision(
                                reason="one-hot uint16 sum, values <= 255"):
                            nc.vector.reduce_sum(
                                out=sel[:],
                                in_=prod[:, :].rearrange("p (j k) -> p j k", j=8),
                                axis=X)
                        sels.append(sel)
                    idx16 = tkpool.tile([128, 8], U16, tag="idx16")
                    nc.vector.tensor_scalar(
                        out=idx16[:], in0=sels[0][:], scalar1=256, scalar2=None,
                        op0=OP.mult)
                    nc.vector.tensor_tensor(
                        out=idx16[:], in0=idx16[:], in1=sels[1][:], op=OP.add)
                    idx32 = tkpool.tile([128, 8], I32, tag="idx32",
                                        name=f"idx32_{m}_{tb}")
                    nc.vector.tensor_copy(out=idx32[:], in_=idx16[:])

                    # per-head softmax over the encoded top-8 scores
                    rmax = tkpool.tile([128, 1], F32, tag="rmax")
                    nc.vector.reduce_max(out=rmax[:], in_=v8[:], axis=X)
                    ex = tkpool.tile([128, 8], F32, tag="ex")
                    nc.vector.tensor_scalar(
                        out=ex[:], in0=v8[:], scalar1=rmax[:, 0:1], scalar2=None,
                        op0=OP.subtract)
                    nc.scalar.activation(out=ex[:], in_=ex[:], func=AF.Exp)
                    rsum = tkpool.tile([128, 1], F32, tag="rsum")
                    nc.vector.reduce_sum(out=rsum[:], in_=ex[:], axis=X)
                    rinv = tkpool.tile([128, 1], F32, tag="rinv")
                    nc.vector.reciprocal(out=rinv[:], in_=rsum[:])
                    w8 = tkpool.tile([128, 8], F32, tag="w8",
                                     name=f"w8_{m}_{tb}")
                    nc.vector.tensor_scalar(
                        out=w8[:], in0=ex[:], scalar1=rinv[:, 0:1], scalar2=None,
                        op0=OP.mult)
                    routed.append((idx32, w8))
                return routed

            def gathers(m, routed):
                pages = {}
                order = [(j, tb) for j in range(8) for tb in range(TB)]
                if m == N_HEADS - 1:
                    order = [(j, tb) for tb in range(TB) for j in range(8)]
                for j, tb in order:
                        idx32, _ = routed[tb]
                        col = m * 8 + j
                        page = wbp.tile([128, 2 * D], BF16, tag="wbpage",
                                        name=f"pg{tb}_{col}")
                        nc.gpsimd.indirect_dma_start(
                            out=page[:], out_offset=None,
                            in_=wb_d[:, :],
                            in_offset=IndirectOffsetOnAxis(
                                ap=idx32[:, j:j + 1], axis=0),
                        )
                        pages[(tb, j)] = page
                return pages

            def compute(m, routed, pages):
                gs = slice(m * 8, (m + 1) * 8)
                order = [(j, tb) for j in range(8) for tb in range(TB)]
                if m == N_HEADS - 1:
                    order = [(j, tb) for tb in range(TB) for j in range(8)]
                for j, tb in order:
                        col = m * 8 + j
                        dummy = scrp.tile([128, 1], BF16, tag="scr")
                        nc.vector.scalar_tensor_tensor(
                            out=dummy.broadcast_to([128, D]),
                            in0=pages[(tb, j)][:, 0:D], scalar=1.0,
                            in1=xbf_sb[tb], op0=OP.mult, op1=OP.mult,
                            accum_out=inner[tb][:, col:col + 1],
                        )
                for tb in range(TB):
                    _, w8 = routed[tb]
                    rl8 = tkpool.tile([128, 8], F32, tag="rl8")
                    nc.scalar.activation(
                        out=rl8[:], in_=inner[tb][:, gs], func=AF.Relu)
                    va8 = tkpool.tile([128, 8], F32, tag="va8",
                                      name=f"va8_{m}_{tb}")
                    nc.vector.tensor_tensor(
                        out=va8[:], in0=rl8[:], in1=w8[:], op=OP.mult)
                    for j in range(8):
                        col = m * 8 + j
                        diag = dgp.tile([128, 128], BF16, tag="diag")
                        nc.scalar.activation(
                            out=diag[:], in_=id01_sb[:], func=AF.Copy,
                            scale=va8[:, j:j + 1])
                        nc.tensor.matmul(
                            out=pacc[tb][:], lhsT=diag[:],
                            rhs=pages[(tb, j)][:, D:2 * D],
                            start=(col == 0), stop=(col == 63))

            routed_m = route(0)
            routed_n = route(1)
            for m in range(N_HEADS):
                pages_m = gathers(m, routed_m)
                routed_2 = route(m + 2) if m + 2 < N_HEADS else None
                compute(m, routed_m, pages_m)
                routed_m = routed_n
                routed_n = routed_2

            for tb in range(TB):
                acc_sb = accp.tile([128, D], F32, tag=f"acc{tb}",
                                   name=f"acc{tb}")
                nc.vector.tensor_copy(out=acc_sb[:], in_=pacc[tb][:])
                nc.sync.dma_start(
                    out=out_d[tb * 128:(tb + 1) * 128, :], in_=acc_sb[:])

    nc.compile()
    return nc


_NC_CACHE = None


def _get_nc():
    global _NC_CACHE
    if _NC_CACHE is None:
        _NC_CACHE = build_nc()
    return _NC_CACHE


def _prep_in_maps(inputs):
    import ml_dtypes
    q = np.ascontiguousarray(np.asarray(inputs["queries"], dtype=np.float32))
    Wq = np.ascontiguousarray(np.asarray(inputs["Wq"], dtype=np.float32))
    bq = np.asarray(inputs["bq"], dtype=np.float32)
    keys = np.asarray(inputs["keys"], dtype=np.float32)
    wd = np.asarray(inputs["w_down"], dtype=np.float32)
    wu = np.asarray(inputs["w_up"], dtype=np.float32)
    wb = np.ascontiguousarray(
        np.concatenate([wd, wu], axis=1).astype(ml_dtypes.bfloat16))
    id01 = np.eye(128, dtype=np.float32).astype(ml_dtypes.bfloat16)
    iota8 = np.tile(np.arange(8, dtype=np.uint16), (128, 1))

    x = q.reshape(B, D)
    # bqp[p, m] = bq[m*128 + p]
    bqp = np.ascontiguousarray(bq.reshape(N_HEADS, D_KEYS).T)
    # kp{1,2}[c, m, n] = keys[m, half, n, c]
    kp1 = np.ascontiguousarray(keys[:, 0].transpose(2, 0, 1))
    kp2 = np.ascontiguousarray(keys[:, 1].transpose(2, 0, 1))
    # wq repacked [p, k, f] = Wq[k*128+p, f]; head-0 columns split out
    wq_r = Wq.reshape(4, 128, N_HEADS * D_KEYS).transpose(1, 0, 2)
    wqh0 = np.ascontiguousarray(wq_r[:, :, 0:128])
    wqr = np.ascontiguousarray(wq_r[:, :, 128:])

    in_maps = []
    for c in range(N_CORES):
        xc = x[c * BC:(c + 1) * BC]
        # xt [p, k, t] = xc.T[k*128+p, t]
        xt_r = np.ascontiguousarray(
            xc.T.reshape(4, 128, BC).transpose(1, 0, 2))
        xbf_r = np.ascontiguousarray(
            xc.reshape(TB, 128, D).transpose(1, 0, 2)
            .astype(ml_dtypes.bfloat16))
        in_maps.append({
            "xt": xt_r,
            "xbf": xbf_r,
            "wqh0": wqh0,
            "wqr": wqr,
            "bqp": bqp,
            "kp1": kp1,
            "kp2": kp2,
            "wb": wb,
            "id01": id01,
            "iota8": iota8,
        })
    return in_maps


def run(inputs, trace=False):
    """Run on 8 NeuronCores; returns (out [2,1024,512], BassKernelResults)."""
    nc = _get_nc()
    in_maps = _prep_in_maps(inputs)
    res = run_bass_kernel_spmd(
        nc, in_maps, core_ids=list(range(N_CORES)), trace=trace)
    out = np.concatenate(
        [res.results[c]["out"] for c in range(N_CORES)], axis=0)
    return out.reshape(2, 1024, D), res


def kernel(**inputs) -> np.ndarray:
    out, _ = run(inputs, trace=False)
    return out
